# revision 6
# baseline (speedup 1.0000x reference)
"""ClusterNorm1d kernel for Trainium2 (Bass/Tile), 8-core data parallel.

out[b,d,k] = sum_e Std_inv[k,d,e] * (x[b,e,k] - mu[e,k])

Strategy (v4, fp8 DoubleRow correction kernel):
  - Decompose Std_inv = I + R (R ~ 0.01, small). The device computes only
    the *correction* c = 8*[R @ (x - mu) - mu]; the host forms
    out = x + c/8 from its exact f32 copy of x. The identity term never
    moves through the device, so both streams are fp8: x in e4m3 (feeds
    only the R-term; quantization contributes ~1e-3), correction out in
    e3m4 (|c| <= ~6 < 15.5 max normal). Measured end-to-end rel err
    ~6e-3 against the f32 reference (budget 2e-2).
  - Shard batch B=8192 across 8 cores (1024 rows each).
  - fp8 DoubleRow matmul (2 rows/cycle) needs the contraction laid out as
    [64 partitions x 2-in-free]: psum[n,b] = sum_e sum_i W[e,i,n]*xt[e,i,b].
    Pairs (j, j+64) contract together; pairs 0-31 live on partitions 0:64,
    pairs 32-63 on partitions 64:128 (64-partition DMAs only engage the
    8 even/odd SDMA engines, so chunks alternate halves to keep all 16
    busy). Layouts:
      xt[64h+e, i, jj, b] = x[b, e, (32h+jj) + 64i]      (fp8e4)
      W[64h+e, i, jj, n=2d+q] = 8R[(32h+jj)+64q, d, e]*(i==q)  (fp8e4)
      nb[n=2d+q, j] = -8*(R@mu + mu)[d, j+64q]           (f32)
  - Per 512-row segment: one DoubleRow matmul into a 1-bank PSUM tile
    (8 tiles in rotation -> deep pipeline), then a PSUM->SBUF drain fused
    with the per-partition bias, interleaved 5:4 across ACT and DVE
    (~1.2 vs ~1.5 ns/elem), writing fp8e3.
  - DMA: x stream on the SP HWDGE ring (pure, no other entries); W and
    the out stream on gpsimd SWDGE so the ACT engine spends no time
    issuing DMAs. ~17.9 MB/core total vs ~36 MB for bf16-everything.
"""

import numpy as np

B, D, K = 8192, 64, 128
N_CORES = 8
B_SHARD = B // N_CORES  # 1024
P = 128                 # SBUF partitions
NPAIR = K // 2          # 64 cluster pairs: (j, j+64)
NHALF = NPAIR // 2      # 32 pairs per partition-half
SEG = 512               # matmul moving free-dim per PSUM bank
CSCALE = 8.0            # correction scale: device returns 8*c, host divides

# x chunk schedule: (half, start_pair_within_half, n_pairs), alternating
# halves so both SDMA engine sets stay busy; smaller tail chunks so the
# pipeline drains early. Consecutive H0/H1 entries share one 128-partition
# tile (H0 in partitions 0:64, H1 in 64:128).
IN_COUPLES = [8, 8, 8, 4, 2, 2]     # pairs per half per couple (sums to 32)
OUT_CHUNK = 4                        # pairs per out DMA entry (processing order)

_cache = {}


def _proc_order():
    """Pair processing order: alternate 8-pair groups between halves."""
    order = []
    s = 0
    for n in IN_COUPLES:
        order.append(("h0", s, n))
        order.append(("h1", s, n))
        s += n
    return order


def _build_nc(b_shard):
    import concourse.tile as tile
    from concourse import bacc, mybir

    f32 = mybir.dt.float32
    bf16 = mybir.dt.bfloat16
    fp8e4 = mybir.dt.float8e4
    fp8e3 = mybir.dt.float8e3
    DR = mybir.MatmulPerfMode.DoubleRow
    nc = bacc.Bacc("TRN2", target_bir_lowering=False)

    xt_d = nc.dram_tensor("xt", [P, 2, NHALF, b_shard], fp8e4,
                          kind="ExternalInput")
    w_d = nc.dram_tensor("w", [P, 2, NHALF, P], fp8e4, kind="ExternalInput")
    nb_d = nc.dram_tensor("nbias", [P, NPAIR], f32, kind="ExternalInput")
    o_d = nc.dram_tensor("out", [P, NPAIR, b_shard], fp8e3,
                         kind="ExternalOutput")

    seg = min(SEG, b_shard)
    nseg = b_shard // seg

    with tile.TileContext(nc) as tc:
        with (
            tc.tile_pool(name="consts", bufs=1) as consts,
            tc.tile_pool(name="xin", bufs=3) as xin,
            tc.tile_pool(name="oout", bufs=8) as oout,
            tc.tile_pool(name="ps", bufs=8, space="PSUM") as psp,
        ):
            w_sb = consts.tile([P, 2, NHALF, P], fp8e4)
            nb_sb = consts.tile([P, NPAIR], f32)

            # Engine warm-ups on a zeroed local tile — independent of the
            # const DMAs so they run during the queue spin-up.
            warm_in = consts.tile([P, P], bf16)
            nc.gpsimd.memset(warm_in, 0)
            scratch = consts.tile([P, 2], f32)
            nc.gpsimd.memset(scratch, 0)
            warm_ps = psp.tile([P, seg], f32, tag="ps")
            nc.tensor.matmul(warm_ps[:, 0:P], lhsT=warm_in, rhs=warm_in)
            nc.scalar.copy(out=scratch[:, 0:1], in_=scratch[:, 0:1])
            nc.vector.tensor_copy(out=scratch[:, 1:2], in_=scratch[:, 1:2])

            # W rides SWDGE (gpsimd) so the SP ring carries nothing but x
            # and the ACT engine issues no DMAs at all.
            nc.gpsimd.dma_start(out=w_sb, in_=w_d[:])

            # Pair processing order (interleaves halves, matching chunks)
            order = _proc_order()

            # x DMA couples: one 128-partition tile per (H0 chunk, H1 chunk)
            couple_tiles = {}

            def ensure_couple(ci):
                if ci in couple_tiles:
                    return couple_tiles[ci]
                n = IN_COUPLES[ci]
                s = sum(IN_COUPLES[:ci])
                t = xin.tile([P, 2, n, b_shard], fp8e4, tag="xt")
                nc.sync.dma_start(out=t[0:64], in_=xt_d[0:64, :, s:s + n, :])
                nc.sync.dma_start(out=t[64:128],
                                  in_=xt_d[64:128, :, s:s + n, :])
                if ci == 0:
                    nc.sync.dma_start(out=nb_sb, in_=nb_d[:])
                couple_tiles[ci] = (t, s, n)
                return couple_tiles[ci]

            # Prefetch map: pair index (within half) -> couple index
            couple_of = {}
            s = 0
            for ci, n in enumerate(IN_COUPLES):
                for t in range(n):
                    couple_of[s + t] = ci
                s += n

            # 5:4 ACT:DVE interleave for the 2*NPAIR seg drains
            drain_pat = "ADADADADA"

            sidx = 0
            for half, gs, gn in order:
                h = 0 if half == "h0" else 1
                h0 = 64 * h
                # out chunks of OUT_CHUNK pairs, never straddling a
                # processing group (keeps DRAM j ranges contiguous)
                for cs0 in range(0, gn, OUT_CHUNK):
                    out_n = min(OUT_CHUNK, gn - cs0)
                    j0 = 32 * h + gs + cs0
                    o_sb = oout.tile([P, out_n, b_shard], fp8e3, tag="o")
                    for u in range(out_n):
                        jh = gs + cs0 + u     # pair index within half
                        j = 32 * h + jh       # global pair id
                        t, cs, cn = ensure_couple(couple_of[jh])
                        nbj = nb_sb[:, j:j + 1]
                        for hh in range(nseg):
                            ps = psp.tile([P, seg], f32, tag="ps")
                            nc.tensor.matmul(
                                ps, lhsT=w_sb[h0:h0 + 64, :, jh, :],
                                rhs=t[h0:h0 + 64, :, jh - cs,
                                      hh * seg:(hh + 1) * seg],
                                perf_mode=DR)
                            dst = o_sb[:, u, hh * seg:(hh + 1) * seg]
                            if drain_pat[sidx % 9] == "A":
                                nc.scalar.add(dst, ps, nbj)
                            else:
                                nc.vector.tensor_scalar_add(dst, ps, nbj)
                            sidx += 1
                    nc.gpsimd.dma_start(
                        out=o_d[:, j0:j0 + out_n, :], in_=o_sb)

    nc.compile()
    return nc


def _host_prep(mu_track, Std_inv_track):
    """Half-split DoubleRow panels W[64h+e, i, jj, n=2d+q] =
    8R[(32h+jj)+64q, d, e]*(i==q) in fp8e4, and the negated per-partition
    bias nbias[n=2d+p, j] = -8*(R@mu + mu)[d, j+64p] (f32), R = S - I."""
    import ml_dtypes

    S = np.ascontiguousarray(Std_inv_track, dtype=np.float32)
    mu = np.ascontiguousarray(mu_track, dtype=np.float32)
    R = S - np.eye(D, dtype=np.float32)[None]
    R8 = CSCALE * R                                    # [k, d, e]

    W = np.zeros((2, D, 2, NHALF, D, 2), dtype=np.float32)
    # [half, e, i, jj, d, q]; k = 32*half + jj + 64*q
    Rq0 = R8[:NPAIR].reshape(2, NHALF, D, D)           # q=0: [h, jj, d, e]
    Rq1 = R8[NPAIR:].reshape(2, NHALF, D, D)           # q=1
    W[:, :, 0, :, :, 0] = Rq0.transpose(0, 3, 1, 2)    # [h, e, jj, d]
    W[:, :, 1, :, :, 1] = Rq1.transpose(0, 3, 1, 2)
    W = W.reshape(P, 2, NHALF, P)

    bias_dk = np.einsum("kde,ek->dk", R, mu) + mu      # [d, k], k = 64q + j
    nbias = (-CSCALE) * bias_dk.reshape(D, 2, NPAIR).reshape(2 * D, NPAIR)
    return (W.astype(ml_dtypes.float8_e4m3),
            np.ascontiguousarray(nbias, dtype=np.float32))


def _pack_x(x, n_cores, b_shard):
    """x [n_cores*b_shard, D, K] f32 -> xt [n_cores, 128, 2, 32, b_shard]
    fp8e4 with xt[core, 64h+e, i, jj, b] = x[core*b_shard+b, e, 32h+jj+64i]."""
    import ml_dtypes

    xb = np.ascontiguousarray(x, dtype=np.float32).astype(
        ml_dtypes.float8_e4m3)
    xp = xb.reshape(n_cores, b_shard, D, 2, 2, NHALF)  # [c, b, e, i, h, jj]
    xt = xp.transpose(0, 4, 2, 3, 5, 1)                # [c, h, e, i, jj, b]
    return np.ascontiguousarray(xt).reshape(n_cores, P, 2, NHALF, b_shard)


def _unpack_out(oT, x, n_cores, b_shard):
    """oT [n_cores, 128, NPAIR, b_shard] fp8e3 (the scaled correction 8c with
    c[b, d, j+64q] = oT[core, 2d+q, j, b]) -> out = x + c/8 [B, D, K] f32."""
    ov = np.asarray(oT).astype(np.float32)
    ov = ov.reshape(n_cores, D, 2, NPAIR, b_shard)     # [core, d, q, j, b]
    c = ov.transpose(0, 4, 1, 2, 3)                    # [core, b, d, q, j]
    c = np.ascontiguousarray(c).reshape(n_cores * b_shard, D, K)
    return np.asarray(x, dtype=np.float32) + c * (1.0 / CSCALE)


def kernel(x, mu_track, Std_inv_track):
    from concourse.bass_utils import run_bass_kernel_spmd

    xt = _pack_x(x, N_CORES, B_SHARD)
    W, nbias = _host_prep(mu_track, Std_inv_track)

    if "nc" not in _cache:
        _cache["nc"] = _build_nc(B_SHARD)
    nc = _cache["nc"]

    in_maps = []
    for i in range(N_CORES):
        in_maps.append({"xt": xt[i], "w": W, "nbias": nbias})
    res = run_bass_kernel_spmd(nc, in_maps, core_ids=list(range(N_CORES)))
    oT = np.stack([r["out"] for r in res.results], axis=0)
    return _unpack_out(oT, x, N_CORES, B_SHARD)


# revision 7
# speedup vs baseline: 1.0671x; 1.0671x over previous
"""ClusterNorm1d kernel for Trainium2 (Bass/Tile), 8-core data parallel.

out[b,d,k] = sum_e Std_inv[k,d,e] * (x[b,e,k] - mu[e,k])

Strategy (v3, fp8 correction output):
  - Decompose Std_inv = I + R (R ~ 0.01, small). The device computes only
    the *correction* c = 8*[R @ (x - mu) - mu] and the host forms
    out = x + c/8 with its exact f32 copy of x. The identity term never
    moves through the device, so the output stream shrinks to fp8
    (e3m4, 4 mantissa bits; |c| <= ~6 < 15.5 max normal) and the weight
    panels shrink to fp8 (e4m3; values 8*R ~ 0.08..0.4 are all normal).
    Measured end-to-end rel err ~4e-3 against the f32 reference
    (budget 2e-2).
  - Shard batch B=8192 across 8 cores (1024 rows each).
  - Host packs x pre-transposed and pair-interleaved in fp8e4 (x only
    feeds the R-term; the identity term comes from the host's f32 x, so
    fp8 input error contributes ~1e-3):
      xt[c, j, b] = x[b, e, j + 64*p]   with c = 2e + p
    so clusters (j, j+64) share one 128-deep contraction. Weight panels
    are block-diagonal pair panels of 8R in fp8e4:
      W[c=2e+pc, j, n=2d+pd] = 8*R[j+64*pd, d, e] * (pc == pd)
  - Device work per pair j: one stationary-weight matmul (lhsT fp8e4,
    rhs fp8e4 -> psum f32) per 512-row segment, then a PSUM->SBUF drain
    fused with the bias nb[n,j] = -8*(R@mu + mu) (per-partition scalar),
    alternating ACT / DVE, writing fp8e3.
  - DMA: x stream (8.4 MB fp8) on the SP HWDGE queue; W (1 MB) + out
    stream (8.4 MB fp8) on the ACT HWDGE queue. Per-core traffic ~26 MB
    vs ~36 MB for the bf16-everything variant; fabric ceiling is
    ~430 GB/s.
"""

import numpy as np

B, D, K = 8192, 64, 128
N_CORES = 8
B_SHARD = B // N_CORES  # 1024
P = 128                 # SBUF partitions
NPAIR = K // 2          # 64 cluster pairs: (j, j+64)
SEG = 512               # matmul moving free-dim per PSUM bank
CSCALE = 8.0            # correction scale: device returns 8*c, host divides

# DMA chunking (pairs per transfer). Packet size per partition row is
# chunk*b_shard*dtype bytes. Head chunks small so compute starts early,
# tail chunks small so the pipeline drains early.
IN_CHUNKS = [2, 2, 4] + [8] * 6 + [4, 2, 1, 1]
OUT_CHUNKS = [8] * 7 + [4, 2, 1, 1]

_cache = {}


def _bounds(chunks):
    out, s = [], 0
    for c in chunks:
        out.append((s, c))
        s += c
    return out


def _build_nc(b_shard):
    import concourse.tile as tile
    from concourse import bacc, mybir

    f32 = mybir.dt.float32
    bf16 = mybir.dt.bfloat16
    fp8e4 = mybir.dt.float8e4
    fp8e3 = mybir.dt.float8e3
    nc = bacc.Bacc("TRN2", target_bir_lowering=False)

    xt_d = nc.dram_tensor("xt", [P, NPAIR, b_shard], fp8e4, kind="ExternalInput")
    w_d = nc.dram_tensor("w", [P, NPAIR, P], fp8e4, kind="ExternalInput")
    nb_d = nc.dram_tensor("nbias", [P, NPAIR], f32, kind="ExternalInput")
    o_d = nc.dram_tensor("out", [P, NPAIR, b_shard], fp8e3, kind="ExternalOutput")

    seg = min(SEG, b_shard)
    nseg = b_shard // seg
    in_bounds = _bounds(IN_CHUNKS)
    out_bounds = _bounds(OUT_CHUNKS)

    with tile.TileContext(nc) as tc:
        with (
            tc.tile_pool(name="consts", bufs=1) as consts,
            tc.tile_pool(name="xin", bufs=6) as xin,
            tc.tile_pool(name="ps", bufs=4, space="PSUM") as psp,
        ):
            w_sb = consts.tile([P, NPAIR, P], fp8e4)
            nb_sb = consts.tile([P, NPAIR], f32)
            # Output is fully buffered in SBUF (64 KiB/partition in fp8) so
            # drains never wait on an out-DMA to recycle a pool buffer.
            o_sb = consts.tile([P, NPAIR, b_shard], fp8e3)

            # Engine warm-ups on a zeroed local tile — independent of the
            # const DMAs so they run during the queue spin-up.
            warm_in = consts.tile([P, P], bf16)
            nc.gpsimd.memset(warm_in, 0)
            scratch = consts.tile([P, 2], f32)
            nc.gpsimd.memset(scratch, 0)
            warm_ps = psp.tile([P, 2, seg], f32, tag="ps")
            nc.tensor.matmul(warm_ps[:, 0, 0:P], lhsT=warm_in, rhs=warm_in)
            nc.scalar.copy(out=scratch[:, 0:1], in_=scratch[:, 0:1])
            nc.vector.tensor_copy(out=scratch[:, 1:2], in_=scratch[:, 1:2])

            # HWDGE descriptor generation is ~24ns/descriptor (~3.1us per
            # 128-row DMA entry) and serializes per queue. The SP ring leads
            # with the first x chunks; the whole fp8 W panel (1 MB) rides the
            # ACT ring as a single entry ahead of the out stream.
            nc.scalar.dma_start(out=w_sb, in_=w_d[:])

            in_it = iter(in_bounds)
            out_it = iter(out_bounds)
            xt = None
            in_s = in_n = 0
            out_s, out_n = next(out_it)
            for j in range(NPAIR):
                if xt is None or j >= in_s + in_n:
                    in_s, in_n = next(in_it)
                    xt = xin.tile([P, in_n, b_shard], fp8e4, tag="xt")
                    nc.sync.dma_start(
                        out=xt, in_=xt_d[:, in_s:in_s + in_n, :])
                    if j == 0:
                        nc.sync.dma_start(out=nb_sb, in_=nb_d[:])
                # both halves of pair j land in one 2-bank PSUM tile, then
                # drain in a single bias-fused op (alternating ACT/DVE)
                ps = psp.tile([P, nseg, seg], f32, tag="ps")
                for h in range(nseg):
                    nc.tensor.matmul(
                        ps[:, h, :], lhsT=w_sb[:, j, :],
                        rhs=xt[:, j - in_s, h * seg:(h + 1) * seg])
                dst = o_sb[:, j, :]
                src = ps.rearrange("p a b -> p (a b)")
                nbj = nb_sb[:, j:j + 1]
                if j % 2 == 0:
                    nc.scalar.add(dst, src, nbj)
                else:
                    nc.vector.tensor_scalar_add(dst, src, nbj)
                if j == out_s + out_n - 1:
                    nc.scalar.dma_start(
                        out=o_d[:, out_s:out_s + out_n, :],
                        in_=o_sb[:, out_s:out_s + out_n, :])
                    if j < NPAIR - 1:
                        out_s, out_n = next(out_it)

    nc.compile()
    return nc


def _host_prep(mu_track, Std_inv_track):
    """Block-diagonal pair panels of the residual W[c=2e+pc, j, n=2d+pd] =
    8*R[j+64pd, d, e]*(pc==pd) in fp8e4, and the negated per-partition bias
    nbias[n=2d+p, j] = -8*(R@mu + mu)[d, j+64p] (f32), where R = S - I."""
    import ml_dtypes

    S = np.ascontiguousarray(Std_inv_track, dtype=np.float32)
    mu = np.ascontiguousarray(mu_track, dtype=np.float32)
    R = S - np.eye(D, dtype=np.float32)[None]

    W = np.zeros((2 * D, NPAIR, 2 * D), dtype=np.float32)
    W6 = W.reshape(D, 2, NPAIR, D, 2)                 # [e, pc, j, d, pd]
    R_r = (CSCALE * R).reshape(2, NPAIR, D, D)        # [pk, j, d, e]
    W6[:, 0, :, :, 0] = R_r[0].transpose(2, 0, 1)     # [e, j, d]
    W6[:, 1, :, :, 1] = R_r[1].transpose(2, 0, 1)

    bias_dk = np.einsum("kde,ek->dk", R, mu) + mu     # [d, k], k = 64p + j
    nbias = (-CSCALE) * bias_dk.reshape(D, 2, NPAIR).reshape(2 * D, NPAIR)
    return (W.astype(ml_dtypes.float8_e4m3),
            np.ascontiguousarray(nbias, dtype=np.float32))


def _pack_x(x, n_cores, b_shard):
    """x [n_cores*b_shard, D, K] f32 -> xt [n_cores, 128, NPAIR, b_shard] bf16
    with xt[core, 2e+p, j, b] = x[b, e, j + 64p]."""
    import ml_dtypes

    xb = np.ascontiguousarray(x, dtype=np.float32).astype(ml_dtypes.float8_e4m3)
    xp = xb.reshape(n_cores, b_shard, D, 2, NPAIR)    # [core, b, e, p, j]
    xt = xp.transpose(0, 2, 3, 4, 1)                  # [core, e, p, j, b]
    return np.ascontiguousarray(xt).reshape(n_cores, P, NPAIR, b_shard)


def _unpack_out(oT, x, n_cores, b_shard):
    """oT [n_cores, 128, NPAIR, b_shard] fp8e3 (the scaled correction 8c with
    c[b, d, j+64p] = oT[core, 2d+p, j, b]) -> out = x + c/8 [B, D, K] f32."""
    ov = np.asarray(oT).astype(np.float32)
    ov = ov.reshape(n_cores, D, 2, NPAIR, b_shard)    # [core, d, p, j, b]
    c = ov.transpose(0, 4, 1, 2, 3)                   # [core, b, d, p, j]
    c = np.ascontiguousarray(c).reshape(n_cores * b_shard, D, K)
    return np.asarray(x, dtype=np.float32) + c * (1.0 / CSCALE)


def kernel(x, mu_track, Std_inv_track):
    from concourse.bass_utils import run_bass_kernel_spmd

    xt = _pack_x(x, N_CORES, B_SHARD)
    W, nbias = _host_prep(mu_track, Std_inv_track)

    if "nc" not in _cache:
        _cache["nc"] = _build_nc(B_SHARD)
    nc = _cache["nc"]

    in_maps = []
    for i in range(N_CORES):
        in_maps.append({"xt": xt[i], "w": W, "nbias": nbias})
    res = run_bass_kernel_spmd(nc, in_maps, core_ids=list(range(N_CORES)))
    oT = np.stack([r["out"] for r in res.results], axis=0)
    return _unpack_out(oT, x, N_CORES, B_SHARD)
